# revision 1
# baseline (speedup 1.0000x reference)
"""Trainium2 Bass kernel for nn_MCMCSampler.

Math: the energy gradient w.r.t. preds is purely elementwise (the feature
einsum is constant w.r.t. preds, so it drops out of jax.grad):

    p     = sigmoid(x)
    grad  = c * p(1-p) * (w + beta*L),   c[b,h] = mask[b,h]/(horses[b]*V*B)
    x    <- x - STEP*grad*mask

where L = dentropy/dp collapses to exactly x (logit o sigmoid identity; the
eps corrections cancel to O(eps^2/p^2) ~ 1e-13 for |x| <= 0.6). The
per-step update is ~8e-10 against x ~ 0.1, so the gradient is constant
across the 16 steps to ~1e-16: compute delta once from x0, then iterate
subtracts. Odd steps run the plain chain x_t = x_{t-1} - delta (resp.
x_{t-2} - 2*delta) on GPSIMD; even steps run x_t = x_{t-2} - 2*delta on
DVE. Both match the reference scan to ~1 ulp.

Sharding: data-parallel over V (64 variants / 8 cores); no cross-core
communication. Per-core output is [16, 8*1024*24] f32 = 12.6 MB so the
kernel is output-DMA-bound (~35 us at ~360 GB/s). Structure: columns are
split into chunks (smallest first) so the first bytes reach the DMA
engines early; steps are grouped per chunk so early DMAs are small (the
stream starts early) and late ones are big (amortized); out-DMAs alternate
between the SP and ACT HWDGE issuers so descriptor generation overlaps the
previous transfer.
"""

import numpy as np
from contextlib import ExitStack

import concourse.bass as bass
from concourse import bacc
import concourse.mybir as mybir
import concourse.tile as tile
from concourse.bass_utils import run_bass_kernel_spmd

NCORES = 8
V, B, H = 64, 1024, 24
S = 16
STEP_SIZE = 0.1
BETA = 0.1
VSH = V // NCORES          # 8 variants per core
N = VSH * B * H            # 196608 elements per core
P = 128                    # SBUF partitions
F = N // P                 # 1536 free-dim elements per partition

# --- schedule configuration (tunable) ---
FCS = [256, 512, 768]              # column chunk widths (sum == F)
GROUPS_C = [                       # steps per out-DMA group, per chunk
    [1, 1, 2, 4, 8],
    [2, 2, 4, 8],
    [2, 2, 4, 4, 4],
]
ODD_ON_POOL = [True, True, True]   # odd-step chain engine per chunk

NCH = len(FCS)
assert sum(FCS) == F
assert all(sum(g) == S for g in GROUPS_C)

_prog_cache: dict = {}


def _slab_layout():
    """(chunk, tier, step_offset, group_size, dram_elem_offset) per slab in
    DMA-emission order (tier-major, then chunk)."""
    slabs = []
    off = 0
    ntiers = max(len(g) for g in GROUPS_C)
    for k in range(ntiers):
        for c in range(NCH):
            if k >= len(GROUPS_C[c]):
                continue
            gs = GROUPS_C[c][k]
            o = sum(GROUPS_C[c][:k])
            slabs.append((c, k, o, gs, off))
            off += P * gs * FCS[c]
    assert off == S * P * F
    return slabs


def _build_program(w: float, uniform_c: float | None):
    nc = bacc.Bacc("TRN2", target_bir_lowering=False, debug=False)
    x_in = nc.declare_dram_parameter("x0", [P, F], mybir.dt.float32, isOutput=False)
    coef_in = None
    if uniform_c is None:
        coef_in = nc.declare_dram_parameter(
            "coef", [P, F], mybir.dt.float32, isOutput=False
        )
    out = nc.declare_dram_parameter(
        "out", [S * P * F], mybir.dt.float32, isOutput=True
    )

    f32 = mybir.dt.float32
    Act = mybir.ActivationFunctionType
    Alu = mybir.AluOpType

    slabs = _slab_layout()
    cstart = [sum(FCS[:c]) for c in range(NCH)]

    with ExitStack() as ctx:
        tc = ctx.enter_context(tile.TileContext(nc))
        cpool = ctx.enter_context(tc.tile_pool(name="const", bufs=1))
        gpool = ctx.enter_context(tc.tile_pool(name="groups", bufs=1))

        # chunked input loads (SP HWDGE)
        x0, coef = [], []
        for c in range(NCH):
            t = cpool.tile([P, FCS[c]], f32, name=f"x0_{c}", tag=f"x0_{c}")
            nc.sync.dma_start(t[:], x_in[:, cstart[c] : cstart[c] + FCS[c]])
            x0.append(t)
        if uniform_c is None:
            for c in range(NCH):
                t = cpool.tile([P, FCS[c]], f32, name=f"coef_{c}", tag=f"coef_{c}")
                nc.sync.dma_start(t[:], coef_in[:, cstart[c] : cstart[c] + FCS[c]])
                coef.append(t)

        # prologue per chunk: p, p^2 on ACT; u, u2, delta, delta2 on DVE
        dm, dm2 = [], []
        for c in range(NCH):
            fc = FCS[c]
            pc = cpool.tile([P, fc], f32, name=f"p_{c}", tag=f"p_{c}")
            nc.scalar.activation(pc[:], x0[c][:], Act.Sigmoid)
            p2c = cpool.tile([P, fc], f32, name=f"p2_{c}", tag=f"p2_{c}")
            nc.scalar.activation(p2c[:], pc[:], Act.Square)

            uc = cpool.tile([P, fc], f32, name=f"u_{c}", tag=f"u_{c}")
            u2c = cpool.tile([P, fc], f32, name=f"u2_{c}", tag=f"u2_{c}")
            if uniform_c is not None:
                cs = STEP_SIZE * uniform_c
                nc.vector.tensor_scalar(
                    uc[:], x0[c][:], float(cs * BETA), float(cs * w),
                    Alu.mult, Alu.add,
                )
                nc.vector.tensor_scalar(
                    u2c[:], x0[c][:], float(2 * cs * BETA), float(2 * cs * w),
                    Alu.mult, Alu.add,
                )
            else:
                nc.vector.tensor_scalar(
                    uc[:], x0[c][:], float(BETA), float(w), Alu.mult, Alu.add
                )
                nc.vector.tensor_mul(uc[:], uc[:], coef[c][:])
                nc.vector.tensor_scalar_mul(u2c[:], uc[:], 2.0)

            dsc = cpool.tile([P, fc], f32, name=f"ds_{c}", tag=f"ds_{c}")
            nc.vector.tensor_sub(dsc[:], pc[:], p2c[:])
            dmc = cpool.tile([P, fc], f32, name=f"dm_{c}", tag=f"dm_{c}")
            nc.vector.tensor_mul(dmc[:], dsc[:], uc[:])
            dm2c = cpool.tile([P, fc], f32, name=f"dm2_{c}", tag=f"dm2_{c}")
            nc.vector.tensor_mul(dm2c[:], dsc[:], u2c[:])
            dm.append(dmc)
            dm2.append(dm2c)

        # group tiles: gt[c][k] is [P, gs*fc]; column j holds step o+j+1
        gt = [
            [gpool.tile([P, gs * FCS[c]], f32, name=f"g_{c}_{k}", tag=f"g_{c}_{k}")
             for k, gs in enumerate(GROUPS_C[c])]
            for c in range(NCH)
        ]

        def step_ap(c, t):
            """AP holding step t (1-indexed) of chunk c; t==0 -> x0."""
            if t == 0:
                return x0[c][:]
            k, o = 0, 0
            while o + GROUPS_C[c][k] < t:
                o += GROUPS_C[c][k]
                k += 1
            j = t - 1 - o
            fc = FCS[c]
            return gt[c][k][:, j * fc : (j + 1) * fc]

        # subtract chains, emitted tier-major so early groups finish first
        dma_i = 0
        for c, k, o, gs, off in slabs:
            for j in range(gs):
                t = o + j + 1
                dst = step_ap(c, t)
                odd_eng = nc.gpsimd if ODD_ON_POOL[c] else nc.vector
                if t == 1:
                    odd_eng.tensor_sub(dst, x0[c][:], dm[c][:])
                elif t == 2:
                    nc.vector.tensor_sub(dst, x0[c][:], dm2[c][:])
                elif t % 2 == 1:
                    odd_eng.tensor_sub(dst, step_ap(c, t - 2), dm2[c][:])
                else:
                    nc.vector.tensor_sub(dst, step_ap(c, t - 2), dm2[c][:])
            dst = out[off : off + P * gs * FCS[c]].rearrange("(p x) -> p x", p=P)
            issuer = nc.sync if (k == 0 or dma_i % 2 == 0) else nc.scalar
            issuer.dma_start(dst, gt[c][k][:])
            dma_i += 1

    nc.compile()
    return nc


def kernel(features, predictions_init, W_feat, w_prob, b, attention_mask):
    preds = np.ascontiguousarray(predictions_init, dtype=np.float32)
    mask = attention_mask.astype(np.float32)
    horses = mask.sum(axis=-1)                       # [B]
    c = (mask * mask) / (horses[:, None] * (V * B))  # [B,H]
    w = float(np.asarray(w_prob).reshape(-1)[0])

    c0 = float(c.flat[0])
    uniform = bool(np.all(c == c0))

    key = (w, c0 if uniform else None)
    if key not in _prog_cache:
        _prog_cache[key] = _build_program(w, c0 if uniform else None)
    nc = _prog_cache[key]

    in_maps = []
    for core in range(NCORES):
        shard = preds[core * VSH : (core + 1) * VSH].reshape(P, F)
        m = {"x0": np.ascontiguousarray(shard)}
        if not uniform:
            ctile = np.broadcast_to(c[None] * 1.0, (VSH, B, H)).reshape(P, F)
            m["coef"] = np.ascontiguousarray(ctile, dtype=np.float32)
        in_maps.append(m)

    res = run_bass_kernel_spmd(nc, in_maps, core_ids=list(range(NCORES)))

    slabs = _slab_layout()
    cstart = [sum(FCS[:c]) for c in range(NCH)]
    outs = []
    for r in res.results:
        arr = r["out"]
        result = np.empty((S, P, F), dtype=np.float32)
        for c, k, o, gs, off in slabs:
            fc = FCS[c]
            block = arr[off : off + P * gs * fc].reshape(P, gs, fc)
            result[o : o + gs, :, cstart[c] : cstart[c] + fc] = (
                block.transpose(1, 0, 2)
            )
        outs.append(result.reshape(S, VSH, B, H))
    full = np.concatenate(outs, axis=1)              # [S, V, B, H]
    return full[..., None].astype(np.float32)

